# revision 51
# baseline (speedup 1.0000x reference)
"""ChebConv (K=5) Trainium2 Bass kernel, 8-core SPMD — monomial form,
all-fp8 DoubleRow applies with residual compensation.

Math: out = x0 @ C0 + sum_{j=1..4} L^j @ (x0 @ Cj) + bias (monomial
coefficients from the Chebyshev expansion).  The four L-power applies
ride fp8e4 DoubleRow pairs at 2x PE rate; because raw f8 quantization of
the L1/L2 terms costs ~3e-2 relative error, those two terms are
compensated with an f8 high/low split on BOTH operands:

  P1: (L1h, L2h) x (c1h, c2h)        main terms
  P2: (L1l, L2l) x (c1h, c2h)        L-residual correction
  P3: (L1h/16, L2h/16) x (16*(c1-c1h), 16*(c2-c2h))   c-residual corr.
  P4: (S3 L3, S4 L4) x (c3, c4)      raw f8 (insensitive terms)

which drops only the (L-resid x c-resid) cross term (CPU mirror of the
full pipeline: rel err 3.9e-3 vs the 2e-2 gate).

Per-core: D sharded by (b, x-plane-pair): core i handles b=i//4,
x in {2*(i%4), 2*(i%4)+1} -> 128 xyz positions * 32 fin = 4096 columns,
processed in 8 chunks of 512.  Chunks are software-pipelined: chunk c's
GEMM phase is emitted before chunk c-1's apply phase so the f8 stream
evacuation latency hides under apply matmuls, and every PSUM pool is
multi-buffered (pm12 x3 + pm34 x2 + po x3 = 8 banks exactly).

Engines: PE GEMMs + applies (the apply psum group is opened by a
full-width ones/128 x bias matmul, so bias costs no vector work); DVE
evacuates c1h/c2h (f32->f8) and computes the c-residuals (STT
psum-minus-f8, same queue so the GEMM psum frees fast); ACT scales
residuals x16 into f8 (activation-copy with scale), evacuates c3/c4,
and copies out tiles.  gpsimd is untouched: its ops cost ~2.5us each on
real HW (vs ~0.45us in the cost model) and cannot read PSUM.

evac=6 refinement: the c1/c2 streams are stored pre-scaled by RS=4
(one DVE tensor_scalar), so the residual STT writes f8 directly and
P1/P3 share one (L/RS) stationary pair tensor — no ACT scale-stores at
all.  Engine balance (sim): PE 86%, DVE 70%, ACT 50%.

Measured (unroll-diff method, see bench.py; median estimator): 229us
per kernel vs 365us for the 241573ns-graded baseline; TimelineSim
single-shot 206.3us vs 341us (constants-DMA reorder removed the 13us
startup stall: wg + chunk-0 xt now land before the lp pair tensors).
HW rel err 4.6e-3 (gate 2e-2).
The xt chunk DMA is prefetched one whole block early so it sits ahead
of the out-DMAs in SP program order (no head-of-line blocking on the
last out tile).  Tested-and-rejected on HW: gpsimd anywhere
(~2.5us/op), 512-col DR moving (+78us), pm5/po3 psum split (+78us),
cs bufs=3 (+81us).  NOTE: cross-process HW timing drifts +-15% on this
box; only same-process tK comparisons are trustworthy.
"""

import numpy as np
import ml_dtypes

B, FIN, V, X, Y, Z = 2, 32, 768, 8, 8, 8
K, FOUT = 5, 32
XYZ = X * Y * Z
NCORES = 8
XZL = 128            # xyz positions per core (2 x-planes * 64)
DLOC = XZL * FIN     # 4096 columns per core
VT = V // 128        # 6 vertex partition tiles
CH = 512             # out columns per chunk = 4 groups * (4 xz * 32 fout)
NCH = DLOC // CH     # 8 chunks
G = 4                # (xz4, f32) groups per chunk
S1, S2, S3, S4 = 32.0, 32.0, 32.0, 128.0  # fp8 balance scales
RS = 4.0             # c-stream upscale (stationaries hold L/RS)
EVAC = 6             # default evac scheme (see _build_nc)

_cache = {}


def _build_nc(reps=1, evac=None, dr512=False, pmb=6, pob=2, csb=2):
    # evac: 4 = DVE (c12h copy, sub) + ACT (xRS store, c34, out)
    #       6 = pre-scaled streams: slots01 = f8(RS*c12) via one DVE
    #           tensor_scalar, slots23 = f8(RS*c - slots01) via one DVE
    #           STT writing f8 directly; stationaries hold L/RS so P1 and
    #           P3 share one pair tensor.  ACT only does c34 + out copies.
    #      11 = minus 3 DR apply passes (timing probe, wrong results)
    # dr512: single 512-col DR apply matmuls instead of 2x256 h-split
    if evac is None:
        evac = EVAC
    import concourse.bacc as bacc
    import concourse.mybir as mybir
    from concourse.tile import TileContext

    f32 = mybir.dt.float32
    bf16 = mybir.dt.bfloat16
    f8 = mybir.dt.float8e4
    DR = mybir.MatmulPerfMode.DoubleRow
    sub = mybir.AluOpType.subtract
    mult = mybir.AluOpType.mult
    add = mybir.AluOpType.add

    nc = bacc.Bacc(None, target_bir_lowering=False)
    xt_d = nc.declare_dram_parameter("xt", [128, NCH, G, V], bf16,
                                     isOutput=False)
    lp_d = [nc.declare_dram_parameter(f"lp{k}", [128, VT * VT, 2, 128], f8,
                                      isOutput=False) for k in range(4)]
    wg_d = nc.declare_dram_parameter("wg", [128, 5, 128], bf16,
                                     isOutput=False)
    # ones/128 stationary + bf16 bias-pattern moving: the apply-psum opener
    # matmul computes the bias broadcast, so out evac is a plain copy.
    ones_d = nc.declare_dram_parameter("ones", [128, 128], bf16,
                                       isOutput=False)
    bias_d = nc.declare_dram_parameter("biasb", [128, CH], bf16,
                                       isOutput=False)
    out_d = nc.declare_dram_parameter("outp", [V, DLOC], f32, isOutput=True)

    with TileContext(nc) as tc:
        with (
            tc.tile_pool(name="consts", bufs=1) as cpool,
            tc.tile_pool(name="xtp", bufs=3) as xtpool,
            tc.tile_pool(name="csp", bufs=csb) as cspool,
            tc.tile_pool(name="rp", bufs=4) as rpool,
            tc.tile_pool(name="outs", bufs=3) as opool,
            tc.tile_pool(name="pmp", bufs=(3 if evac == 5 else pmb),
                         space="PSUM") as pmpool,
            tc.tile_pool(name="pop", bufs=pob, space="PSUM") as popool,
        ):
            lpidx = [0, 1, 3] if evac == 6 else [0, 1, 2, 3]
            lp = {k: cpool.tile([128, VT * VT, 2, 128], f8, name=f"lp{k}")
                  for k in lpidx}
            wg = cpool.tile([128, 5, 128], bf16)
            ones = cpool.tile([128, 128], bf16)
            biasb = cpool.tile([128, CH], bf16)
            # DMA order: what the first GEMM needs (wg + chunk-0 xt) lands
            # before the 3.5MB of lp pair tensors, which the first apply
            # only needs ~20us in — trims ~10us off single-shot startup.
            nc.sync.dma_start(out=wg[:], in_=wg_d[:])
            xtc0 = xtpool.tile([128, G, V], bf16, tag="xtc", name="xtc0")
            nc.sync.dma_start(out=xtc0[:], in_=xt_d[:, 0, :, :])
            nc.sync.dma_start(out=ones[:], in_=ones_d[:])
            nc.sync.dma_start(out=biasb[:], in_=bias_d[:])
            for k in lpidx:
                nc.sync.dma_start(out=lp[k][:], in_=lp_d[k][:])

            def emit_apply_vt(xtp, cs, cc, vt):
                # apply group for one out tile: PSUM accumulation group
                # opened by a single full-width bias matmul.
                po = popool.tile([128, CH], f32, tag="po")
                nc.tensor.matmul(po[:], ones[:], biasb[:],
                                 start=True, stop=False)
                for g in range(G):
                    nc.tensor.matmul(
                        po[:, g * 128:(g + 1) * 128],
                        xtp[:, g, vt * 128:(vt + 1) * 128],
                        wg[:, 0, :], start=False, stop=False)
                if evac == 6:
                    passes = [(lp[0], 0), (lp[1], 0), (lp[0], 2), (lp[3], 4)]
                else:
                    passes = [(lp[0], 0), (lp[1], 0), (lp[2], 2), (lp[3], 4)]
                if evac == 11:
                    passes = passes[:1]
                for p, (lpt, s0) in enumerate(passes):
                    for vi in range(VT):
                        last = (p == len(passes) - 1 and vi == VT - 1)
                        if dr512:
                            nc.tensor.matmul(
                                po[:], lpt[:, vi * VT + vt, :, :],
                                cs[vi][:, s0:s0 + 2, :],
                                start=False, stop=last, perf_mode=DR)
                        else:
                            for h in range(2):
                                nc.tensor.matmul(
                                    po[:, h * 256:(h + 1) * 256],
                                    lpt[:, vi * VT + vt, :, :],
                                    cs[vi][:, s0:s0 + 2,
                                           h * 256:(h + 1) * 256],
                                    start=False, stop=(last and h == 1),
                                    perf_mode=DR)
                ot = opool.tile([128, CH], f32, tag="ot")
                if evac in (4, 6):
                    nc.scalar.copy(ot[:], po[:])
                else:
                    nc.vector.tensor_copy(ot[:], po[:])
                nc.sync.dma_start(
                    out=out_d[vt * 128:(vt + 1) * 128,
                              cc * CH:(cc + 1) * CH],
                    in_=ot[:])

            prev = None
            total = NCH * reps
            # chunk 0 was prefetched with the constants; each block then
            # prefetches chunk c+1 BEFORE its out-DMAs are queued, so SP
            # never head-blocks the xt load
            xtc = xtc0
            for c in range(total):
                cc = c % NCH
                if c + 1 < total:
                    nxt = xtpool.tile([128, G, V], bf16, tag="xtc",
                                      name="xtc_n")
                    nc.sync.dma_start(out=nxt[:],
                                      in_=xt_d[:, (c + 1) % NCH, :, :])

                # ---- GEMM phase for chunk cc, interleaved per-vi with the
                # previous chunk's apply groups so PE has apply matmuls to
                # chew on while the evac engines drain each pm quad ----
                # cs slots: 0=c1h 1=c2h 2=c1l16 3=c2l16 4=c3 5=c4
                cs = [cspool.tile([128, 6, CH], f8, tag=f"cs{vi}",
                                  name=f"cs{vi}") for vi in range(VT)]
                for vi in range(VT):
                    if evac == 5:
                        for gp in range(2):
                            ps = slice(gp * 256, (gp + 1) * 256)
                            pm2 = pmpool.tile([128, 2, 4, 128], f32,
                                              tag="pm2")
                            for gg in range(2):
                                g = 2 * gp + gg
                                nc.tensor.matmul(
                                    pm2[:, gg, :, :],
                                    xtc[:, g, vi * 128:(vi + 1) * 128],
                                    wg[:, 1:5, :], start=True, stop=True)
                            # src views iterate (j, g, col) to match cs slots
                            s12 = pm2[:, :, 0:2, :].transpose([0, 2, 1, 3])
                            s34 = pm2[:, :, 2:4, :].transpose([0, 2, 1, 3])
                            nc.vector.tensor_copy(cs[vi][:, 0:2, ps], s12)
                            r = rpool.tile([128, 512], f32, tag="r2")
                            nc.vector.scalar_tensor_tensor(
                                r[:], s12, 1.0, cs[vi][:, 0:2, ps],
                                op0=mult, op1=sub)
                            nc.scalar.mul(cs[vi][:, 2:4, ps], r[:], RS)
                            nc.scalar.copy(cs[vi][:, 4:6, ps], s34)
                        if prev is not None:
                            emit_apply_vt(*prev, vi)
                        continue
                    for g in range(G):
                        if evac == 12:
                            break
                        gs = slice(g * 128, (g + 1) * 128)
                        pm = pmpool.tile([128, 4, 128], f32, tag="pm")
                        nc.tensor.matmul(pm[:],
                                         xtc[:, g, vi * 128:(vi + 1) * 128],
                                         wg[:, 1:5, :], start=True, stop=True)
                        if evac == 10:
                            continue
                        if evac == 6:
                            # slots01 = f8(RS*c12); slots23 = f8(RS*c12 -
                            # slots01) in one STT; ACT only copies c34.
                            nc.vector.tensor_scalar_mul(
                                cs[vi][:, 0:2, gs], pm[:, 0:2, :], RS)
                            nc.vector.scalar_tensor_tensor(
                                cs[vi][:, 2:4, gs], pm[:, 0:2, :], RS,
                                cs[vi][:, 0:2, gs], op0=mult, op1=sub)
                            nc.scalar.copy(cs[vi][:, 4:6, gs],
                                           pm[:, 2:4, :])
                            continue
                        if evac in (2, 4):
                            nc.vector.tensor_copy(cs[vi][:, 0:2, gs],
                                                  pm[:, 0:2, :])
                        else:
                            nc.scalar.copy(cs[vi][:, 0:2, gs], pm[:, 0:2, :])
                        r = rpool.tile([128, 256], f32, tag="r")
                        nc.vector.scalar_tensor_tensor(
                            r[:], pm[:, 0:2, :], 1.0,
                            cs[vi][:, 0:2, gs], op0=mult, op1=sub)
                        if evac == 4:
                            nc.scalar.mul(cs[vi][:, 2:4, gs], r[:], RS)
                        else:
                            nc.vector.tensor_scalar_mul(
                                cs[vi][:, 2:4, gs], r[:], RS)
                        if evac == 4 or ((vi + g) % 2 == 0 and evac != 2):
                            nc.scalar.copy(cs[vi][:, 4:6, gs], pm[:, 2:4, :])
                        else:
                            nc.vector.tensor_copy(cs[vi][:, 4:6, gs],
                                                  pm[:, 2:4, :])
                    # previous chunk's apply group vt=vi rides behind this
                    # vi's GEMM quads (SW pipeline, lag one chunk)
                    if prev is not None:
                        emit_apply_vt(*prev, vi)
                prev = (xtc, cs, cc)
                if c + 1 < total:
                    xtc = nxt
            for vt in range(VT):
                emit_apply_vt(*prev, vt)
    nc.compile()
    return nc


def _host_prep(lap_rows, lap_cols, lap_vals, x, weight, bias):
    bf = ml_dtypes.bfloat16
    f8 = ml_dtypes.float8_e4m3fn

    L = np.zeros((V, V), np.float64)
    np.add.at(L, (np.asarray(lap_rows), np.asarray(lap_cols)),
              np.asarray(lap_vals, np.float64))
    L2 = L @ L
    L3 = L2 @ L
    L4 = L2 @ L2

    def blocks(M):
        # [128, (vi, vt), 128] with block (vi,vt) = M.T[vi*128:, vt*128:]
        return (np.asarray(M, np.float32).T
                .reshape(VT, 128, VT, 128).transpose(1, 0, 2, 3))

    def pair(a, b):
        # [128, (vi,vt), 2, 128] f8 stationary DoubleRow pair tensor
        return np.ascontiguousarray(
            np.stack([a, b], axis=3).reshape(128, VT * VT, 2, 128)).astype(f8)

    l1s, l2s = S1 * L, S2 * L2
    l1h = blocks(l1s).astype(f8).astype(np.float32)
    l2h = blocks(l2s).astype(f8).astype(np.float32)
    l1l = blocks(l1s) - l1h
    l2l = blocks(l2s) - l2h
    if EVAC == 6:
        # streams are pre-scaled by RS, stationaries carry the /RS;
        # P1 and P3 share lp0 = (L1h/RS, L2h/RS)
        lp0 = pair(l1h / RS, l2h / RS)
        lp1 = pair(l1l / RS, l2l / RS)
        lp2 = None
    else:
        lp0 = pair(l1h, l2h)                  # P1: main
        lp1 = pair(l1l, l2l)                  # P2: L-residual
        lp2 = pair(l1h / RS, l2h / RS)        # P3: c-residual (xRS streams)
    lp3 = pair(blocks(S3 * L3), blocks(S4 * L4))   # P4: raw

    W = np.asarray(weight, np.float64)
    Cm = [W[0] - W[2] + W[4], (W[1] - 3 * W[3]) / S1,
          (2 * W[2] - 8 * W[4]) / S2, 4 * W[3] / S3, 8 * W[4] / S4]
    wg = np.zeros((5, 128, 128), np.float32)
    for j in range(5):
        for a in range(4):
            wg[j, a * FIN:(a + 1) * FIN, a * FOUT:(a + 1) * FOUT] = Cm[j]
    wg = np.ascontiguousarray(wg.transpose(1, 0, 2)).astype(bf)

    ones = np.full((128, 128), 1.0 / 128.0, np.float32).astype(bf)
    biasb = np.tile(np.asarray(bias, np.float32), (128, CH // FOUT)).astype(bf)

    xf = np.asarray(x, np.float32)
    in_maps = []
    for i in range(NCORES):
        b, xp = i // 4, i % 4
        xsl = xf[b][:, :, 2 * xp:2 * xp + 2]           # [FIN, V, 2, Y, Z]
        # xt[p=(a4,fin32), g, v] with xz = g*4 + a
        xt = np.ascontiguousarray(
            xsl.reshape(FIN, V, XZL).transpose(2, 0, 1)   # [xz, fin, v]
            .reshape(32, 4, FIN, V).transpose(1, 2, 0, 3)  # [a, fin, g, v]
            .reshape(128, NCH, G, V)).astype(bf)
        im = {
            "xt": xt, "lp0": lp0, "lp1": lp1, "lp3": lp3,
            "wg": wg, "ones": ones, "biasb": biasb,
        }
        if lp2 is not None:
            im["lp2"] = lp2
        else:
            im["lp2"] = np.zeros_like(lp0)  # declared but unused param
        in_maps.append(im)
    return in_maps


def _scatter_out(out, outp, i):
    b, xp = i // 4, i % 4
    o = outp.reshape(V, XZL, FOUT)
    o = o.transpose(2, 0, 1).reshape(FOUT, V, 2, Y, Z)
    out[b, :, :, 2 * xp:2 * xp + 2] = o


def kernel(lap_rows, lap_cols, lap_vals, x, weight, bias):
    from concourse.bass_utils import run_bass_kernel_spmd

    if "nc" not in _cache:
        _cache["nc"] = _build_nc()
    nc = _cache["nc"]

    in_maps = _host_prep(lap_rows, lap_cols, lap_vals, x, weight, bias)
    res = run_bass_kernel_spmd(nc, in_maps, core_ids=list(range(NCORES)))

    out = np.empty((B, FOUT, V, X, Y, Z), np.float32)
    for i in range(NCORES):
        _scatter_out(out, res.results[i]["outp"], i)
    return out
